# revision 10
# baseline (speedup 1.0000x reference)
"""DeepSeekMoE Trainium2 kernel: 8-core expert-parallel sparse dispatch.

Strategy (hardcoded for D=5120, F=384, E=32, S=2, T=1024, top-2):
- Host computes the gate (softmax + top-2 + combine weights) and dispatches
  tokens: each of the 8 cores owns 4 routed experts; its assigned tokens are
  gathered, transposed and padded to 96 per expert on the host.
- Routed experts run in fp8 (x, w1 in e4m3 for DoubleRow 2x fc1; act + w2 in
  e3m4). The descales are folded into the per-token combine weights applied
  during the PSUM->SBUF copy. Routed outputs are written in e3m4 (x4 scale):
  they are attenuated ~4.6x by the combine weights, so the extra noise is
  negligible next to the bf16 shared outputs.
- Shared experts stay bf16 (their contribution dominates the output norm).
  Core c handles shared expert c%2 for token quarter c//2.
- Single global DMA read queue (sync engine) issues every load in exact
  consumption order, so early bandwidth goes to the first routed experts
  and the shared tensors stream in just-in-time. Writes go on gpsimd.
- Shared fc1 is split into NG groups interleaved between routed experts:
  each group accumulates 8 k-tiles in a transient PSUM tile (po ring) and
  folds into an SBUF f32 accumulator, so the tensor engine has work while
  routed weights stream and the tail only holds shared SwiGLU + fc2.
- Host gathers: routed outputs are scatter-added via two vectorized gathers,
  shared quarters are summed pairwise.
"""
import sys
import os

sys.path.insert(0, "/opt/trn_rl_repo")

import numpy as np

D = 5120
F = 384
F2 = 768
E = 32
S = 2
T = 1024
NCORE = 8
EPC = E // NCORE          # experts per core
CAP = 96                  # token capacity per expert slot (multiple of 32
                          # required by dual-fp8 ldweights; max load is 85)
QT = T // (NCORE // S)    # tokens per shared quarter = 256
DT = D // 128             # 40 d-tiles
G1 = 8                    # d-tiles per packed group (fc1 inputs)
NG = DT // G1             # 5 groups
FT = F // 128             # 3 f-tiles

KDT = "bf16"   # shared-expert compute dtype
# fp8 scales for the routed path: fc1 in e4m3 (DoubleRow 2x matmul rate),
# act + fc2 weights in e3m4 (better mantissa where it matters)
S_X = 8.0
S_W1 = 128.0
S_W2 = 100.0
S_A = 0.25
S_OR = 4.0     # routed-output e3m4 scale
S_W2S = 100.0  # shared fc2 weight e3m4 scale
S1 = S_X * S_W1
_compiled = {}


def _np_dt(name):
    import concourse.mybir as mybir
    m = {"bf16": mybir.dt.bfloat16, "f32r": mybir.dt.float32r,
         "f8e3": mybir.dt.float8e3, "f8e4": mybir.dt.float8e4}
    return mybir.dt.np(m[name])


def _build(use_b1, use_b2, use_bs1, kdt):
    import concourse.bass as bass
    import concourse.bacc as bacc
    import concourse.tile as tile
    import concourse.mybir as mybir

    F32 = mybir.dt.float32
    F32R = mybir.dt.bfloat16 if kdt == "bf16" else mybir.dt.float32r
    F8 = mybir.dt.float8e3
    F8E4 = mybir.dt.float8e4
    DR = mybir.MatmulPerfMode.DoubleRow
    AF = mybir.ActivationFunctionType

    nc = bacc.Bacc(None, target_bir_lowering=False)

    # ---- DRAM I/O ----
    # routed (fp8)
    xg = nc.dram_tensor("xg", [EPC, NG, 128, G1, CAP], F8E4,
                        kind="ExternalInput")
    w1p = nc.dram_tensor("w1p", [EPC, NG, 128, G1, F2], F8E4,
                         kind="ExternalInput")
    w2p = nc.dram_tensor("w2p", [EPC, FT, 128, D], F8, kind="ExternalInput")
    cwc = nc.dram_tensor("cwc", [CAP, EPC], F32, kind="ExternalInput")
    out_r = nc.dram_tensor("out_r", [EPC, CAP, D], F8, kind="ExternalOutput")
    # shared (bf16; this core's expert s=c%2, token quarter q=c//2)
    xq = nc.dram_tensor("xq", [NG, 128, G1, QT], F32R, kind="ExternalInput")
    w1sp = nc.dram_tensor("w1sp", [NG, 128, G1, F2], F32R, kind="ExternalInput")
    w2sp = nc.dram_tensor("w2sp", [FT, 128, D], F8, kind="ExternalInput")
    out_s = nc.dram_tensor("out_s", [QT, D], F32R, kind="ExternalOutput")
    # constants
    ident = nc.dram_tensor("ident", [128, 128], F32R, kind="ExternalInput")
    if use_b1:
        b1r = nc.dram_tensor("b1r", [EPC, F2], F32R, kind="ExternalInput")
    if use_b2:
        b2r = nc.dram_tensor("b2r", [EPC, D], F32R, kind="ExternalInput")
    if use_bs1:
        b1s = nc.dram_tensor("b1s", [1, F2], F32R, kind="ExternalInput")

    with tile.TileContext(nc) as tc:
        with (
            tc.tile_pool(name="cst", bufs=1) as cst,
            tc.tile_pool(name="res", bufs=1) as res,
            tc.tile_pool(name="wpool", bufs=5) as wpool,
            tc.tile_pool(name="w2pool", bufs=3) as w2pool,
            tc.tile_pool(name="w2spool", bufs=3) as w2spool,
            tc.tile_pool(name="xpool", bufs=5) as xpool,
            tc.tile_pool(name="spool", bufs=2) as spool,
            tc.tile_pool(name="opool", bufs=2) as opool,
            tc.tile_pool(name="ph", bufs=2, space="PSUM") as ph_pool,
            tc.tile_pool(name="pt", bufs=1, space="PSUM") as pt_pool,
            tc.tile_pool(name="po", bufs=3, space="PSUM") as po_pool,
        ):
            ident_t = cst.tile([128, 128], F32R)
            nc.gpsimd.dma_start(ident_t[:], ident[:])
            cw_t = cst.tile([CAP, EPC], F32)
            nc.gpsimd.dma_start(cw_t[:], cwc[:])
            need_ones = use_b1 or use_b2 or use_bs1
            if need_ones:
                ones_t = cst.tile([1, 128], F32R)
                nc.gpsimd.memset(ones_t[:], 1.0)
            if use_b1:
                b1r_t = cst.tile([EPC, F2], F32R)
                nc.gpsimd.dma_start(b1r_t[:], b1r[:])
            if use_b2:
                b2r_t = cst.tile([EPC, D], F32R)
                nc.gpsimd.dma_start(b2r_t[:], b2r[:])
            if use_bs1:
                b1s_t = cst.tile([1, F2], F32R)
                nc.gpsimd.dma_start(b1s_t[:], b1s[:])

            # resident shared-expert inputs (streamed just-in-time on the
            # global read queue) and the SBUF f32 fc1 accumulators
            xq_res = [res.tile([128, G1, QT], F32R, name=f"xqr{k}")
                      for k in range(NG)]
            w1s_res = [res.tile([128, G1, F2], F32R, name=f"w1sr{k}")
                       for k in range(NG)]
            acc_s = [res.tile([128, F2], F32, name=f"accs{t}")
                     for t in range(2)]

            def shared_load(k):
                nc.sync.dma_start(xq_res[k][:], xq[k])
                nc.sync.dma_start(w1s_res[k][:], w1sp[k])

            # ================= routed experts (fp8) =================
            def routed_fc1(e):
                psum_h = ph_pool.tile([CAP, F2], F32, tag="ph", padded_shape=None)
                if use_b1:
                    nc.tensor.matmul(psum_h[:, 0:512], ones_t[:, 0:CAP],
                                     b1r_t[e:e + 1, 0:512], start=True, stop=False)
                    nc.tensor.matmul(psum_h[:, 512:F2], ones_t[:, 0:CAP],
                                     b1r_t[e:e + 1, 512:F2], start=True, stop=False)
                for dtg in range(NG):
                    xg_t = xpool.tile([128, G1, CAP], F8E4, tag="xg")
                    nc.sync.dma_start(xg_t[:], xg[e, dtg])
                    w1_t = wpool.tile([128, G1, F2], F8E4, tag="w1")
                    nc.sync.dma_start(w1_t[:], w1p[e, dtg])
                    first = (dtg == 0) and not use_b1
                    last = dtg == NG - 1
                    for g in range(0, G1, 2):
                        nc.tensor.matmul(psum_h[:, 0:512],
                                         xg_t[:, g:g + 2, :],
                                         w1_t[:, g:g + 2, 0:512],
                                         start=first and g == 0,
                                         stop=(last and g == G1 - 2),
                                         perf_mode=DR)
                        nc.tensor.matmul(psum_h[:, 512:F2],
                                         xg_t[:, g:g + 2, :],
                                         w1_t[:, g:g + 2, 512:F2],
                                         start=first and g == 0,
                                         stop=(last and g == G1 - 2),
                                         perf_mode=DR)
                return psum_h

            def routed_w2_load(e):
                w2_t = [None] * FT
                for ft in range(FT):
                    w2_t[ft] = w2pool.tile([128, D], F8, tag="w2", name="w2t")
                    nc.sync.dma_start(w2_t[ft][:], w2p[e, ft])
                return w2_t

            def routed_finish(e, psum_h, w2_t):
                # SwiGLU: silu(v/S1) * (g*S1), bf16 transpose, then the
                # S_A/S1 descale + e3m4 cast fused into the PSUM->SBUF copy.
                silu_t = spool.tile([128, F], F32, tag="silu")
                nc.scalar.activation(silu_t[:CAP, :], psum_h[:, 0:F], AF.Silu,
                                     scale=1.0 / S1)
                act_t = spool.tile([128, F], F32R, tag="act")
                nc.vector.tensor_mul(act_t[:CAP, :], silu_t[:CAP, :],
                                     psum_h[:, F:F2])
                actT = spool.tile([128, FT, 128], F8, tag="actT")
                for ft in range(FT):
                    ptile = pt_pool.tile([128, 128], F32R)
                    nc.tensor.transpose(
                        ptile[:, 0:CAP], act_t[:CAP, ft * 128:(ft + 1) * 128],
                        ident_t[:CAP, 0:CAP]
                    )
                    nc.scalar.activation(actT[:, ft, 0:CAP], ptile[:, 0:CAP],
                                         AF.Copy, scale=S_A / S1)
                ob = opool.tile([128, D], F8, tag="ob")
                for ch in range(10):
                    po = po_pool.tile([CAP, 512], F32, tag="po", name="po")
                    if use_b2:
                        nc.tensor.matmul(
                            po[:], ones_t[:, 0:CAP],
                            b2r_t[e:e + 1, ch * 512:(ch + 1) * 512],
                            start=True, stop=False)
                    for ft in range(FT):
                        nc.tensor.matmul(
                            po[:], actT[:, ft, 0:CAP],
                            w2_t[ft][:, ch * 512:(ch + 1) * 512],
                            start=(ft == 0) and not use_b2,
                            stop=(ft == FT - 1))
                    nc.scalar.activation(
                        ob[:CAP, ch * 512:(ch + 1) * 512],
                        po[:], AF.Copy, scale=cw_t[:, e:e + 1])
                nc.gpsimd.dma_start(out_r[e], ob[:CAP, :])

            # ====== shared expert fc1, one k-group at a time (bf16) ======
            def shared_fc1_group(dtg):
                # accumulate 8 k-tiles into transient PSUM (po ring), then
                # fold into the SBUF f32 accumulator on vector/scalar
                for tt in range(2):
                    pa = po_pool.tile([128, 512], F32, tag="po", name="po")
                    pb = po_pool.tile([128, 256], F32, tag="po", name="po")
                    if use_bs1 and dtg == 0:
                        nc.tensor.matmul(pa[:], ones_t[:, 0:128],
                                         b1s_t[:, 0:512], start=True, stop=False)
                        nc.tensor.matmul(pb[:], ones_t[:, 0:128],
                                         b1s_t[:, 512:F2], start=True, stop=False)
                    for g in range(G1):
                        st = (g == 0) and not (use_bs1 and dtg == 0)
                        nc.tensor.matmul(
                            pa[:],
                            xq_res[dtg][:, g, tt * 128:(tt + 1) * 128],
                            w1s_res[dtg][:, g, 0:512],
                            start=st, stop=(g == G1 - 1))
                        nc.tensor.matmul(
                            pb[:],
                            xq_res[dtg][:, g, tt * 128:(tt + 1) * 128],
                            w1s_res[dtg][:, g, 512:F2],
                            start=st, stop=(g == G1 - 1))
                    if dtg == 0:
                        nc.vector.tensor_copy(acc_s[tt][:, 0:512], pa[:])
                        nc.scalar.activation(acc_s[tt][:, 512:F2], pb[:],
                                             AF.Copy)
                    else:
                        nc.vector.tensor_add(acc_s[tt][:, 0:512],
                                             acc_s[tt][:, 0:512], pa[:])
                        nc.vector.tensor_add(acc_s[tt][:, 512:F2],
                                             acc_s[tt][:, 512:F2], pb[:])

            # ============ shared expert SwiGLU + fc2 (tail) ============
            def shared_tail():
                w2s_t = [None] * FT
                for ft in range(FT):
                    w2s_t[ft] = w2spool.tile([128, D], F8, tag="w2s",
                                             name="w2st")
                    nc.sync.dma_start(w2s_t[ft][:], w2sp[ft])
                actTs = []
                for tt in range(2):
                    silu_t = spool.tile([128, F], F32, tag="silu")
                    nc.scalar.activation(silu_t[:, :], acc_s[tt][:, 0:F],
                                         AF.Silu)
                    act_t = spool.tile([128, F], F32R, tag="acts")
                    nc.vector.tensor_mul(act_t[:, :], silu_t[:, :],
                                         acc_s[tt][:, F:F2])
                    actT = spool.tile([128, FT, 128], F32R, tag="actTs")
                    for ft in range(FT):
                        ptile = pt_pool.tile([128, 128], F32R)
                        nc.tensor.transpose(
                            ptile[:, :], act_t[:, ft * 128:(ft + 1) * 128],
                            ident_t[:, :]
                        )
                        if ft % 2 == 0:
                            nc.vector.tensor_copy(actT[:, ft, :], ptile[:, :])
                        else:
                            nc.scalar.activation(actT[:, ft, :], ptile[:, :],
                                                 AF.Copy)
                    actTs.append(actT)
                obs = [opool.tile([128, D], F32R, tag="ob", name="obs")
                       for _ in range(2)]
                for tt in range(2):
                    for ch in range(10):
                        po = po_pool.tile([128, 512], F32, tag="po", name="po")
                        for ft in range(FT):
                            nc.tensor.matmul(
                                po[:], actTs[tt][:, ft, :],
                                w2s_t[ft][:, ch * 512:(ch + 1) * 512],
                                start=(ft == 0), stop=(ft == FT - 1))
                        # alternate copy engine so neither backs up at the
                        # tail; fold the w2s e3m4 descale into the copy
                        if ch % 2 == 0:
                            nc.vector.tensor_scalar_mul(
                                obs[tt][:, ch * 512:(ch + 1) * 512], po[:],
                                1.0 / S_W2S)
                        else:
                            nc.scalar.activation(
                                obs[tt][:, ch * 512:(ch + 1) * 512], po[:],
                                AF.Copy, scale=1.0 / S_W2S)
                        if ch == 4:
                            nc.gpsimd.dma_start(
                                out_s[tt * 128:(tt + 1) * 128, 0:2560],
                                obs[tt][:, 0:2560])
                        elif ch == 7:
                            nc.gpsimd.dma_start(
                                out_s[tt * 128:(tt + 1) * 128, 2560:4096],
                                obs[tt][:, 2560:4096])
                    nc.gpsimd.dma_start(
                        out_s[tt * 128:(tt + 1) * 128, 4096:D],
                        obs[tt][:, 4096:D])

            # Software pipeline. All reads are issued on the sync queue in
            # exact consumption order; shared fc1 groups fill the tensor
            # slack while routed weights stream.
            ph = [None] * EPC
            ph[0] = routed_fc1(0)
            ph[1] = routed_fc1(1)
            w2_0 = routed_w2_load(0)
            shared_load(0)
            routed_finish(0, ph[0], w2_0)
            shared_fc1_group(0)
            ph[2] = routed_fc1(2)
            w2_1 = routed_w2_load(1)
            shared_load(1)
            routed_finish(1, ph[1], w2_1)
            shared_fc1_group(1)
            ph[3] = routed_fc1(3)
            w2_2 = routed_w2_load(2)
            shared_load(2)
            routed_finish(2, ph[2], w2_2)
            shared_fc1_group(2)
            w2_3 = routed_w2_load(3)
            shared_load(3)
            routed_finish(3, ph[3], w2_3)
            shared_fc1_group(3)
            shared_load(4)
            shared_fc1_group(4)
            shared_tail()
    nc.compile()
    return nc


def _get_nc(key):
    if key not in _compiled:
        _compiled[key] = _build(*key)
    return _compiled[key]


def _silu(v):
    return v / (1.0 + np.exp(-v))


def _pack_w1(w):  # [D, 2F] -> [NG, 128, G1, 2F]
    return np.ascontiguousarray(
        w.reshape(NG, G1, 128, F2).transpose(0, 2, 1, 3))


def _pack_w2(w):  # [F, D] -> [FT, 128, D]
    return np.ascontiguousarray(w.reshape(FT, 128, D))


def _pack_xT(xt_cols):  # [D, ncols] -> [NG, 128, G1, ncols]
    n = xt_cols.shape[1]
    return np.ascontiguousarray(
        xt_cols.reshape(NG, G1, 128, n).transpose(0, 2, 1, 3))


def _qc(a, s, dt, lim):
    return np.clip(np.asarray(a, np.float32) * s, -lim, lim).astype(dt)


def kernel(x, gate_w, gate_b, shared_w1, shared_b1, shared_w2, shared_b2,
           routed_w1, routed_b1, routed_w2, routed_b2):
    from concourse.bass_utils import run_bass_kernel_spmd

    f32 = np.float32
    x = np.asarray(x, f32)
    gate_w = np.asarray(gate_w, f32)
    gate_b = np.asarray(gate_b, f32)
    shared_w1 = np.asarray(shared_w1, f32)
    shared_b1 = np.asarray(shared_b1, f32)
    shared_w2 = np.asarray(shared_w2, f32)
    shared_b2 = np.asarray(shared_b2, f32)
    routed_w1 = np.asarray(routed_w1, f32)
    routed_b1 = np.asarray(routed_b1, f32)
    routed_w2 = np.asarray(routed_w2, f32)
    routed_b2 = np.asarray(routed_b2, f32)

    B = x.shape[0]
    x2 = x.reshape(T, D)

    # ---- gate: softmax + top-2 (unnormalized combine weights) ----
    logits = x2 @ gate_w + gate_b
    m = logits.max(-1, keepdims=True)
    p = np.exp(logits - m, dtype=f32)
    p = p / p.sum(-1, keepdims=True)
    ar = np.arange(T)
    i1 = np.argmax(p, -1)
    p1 = p[ar, i1]
    pm = p.copy()
    pm[ar, i1] = -1.0
    i2 = np.argmax(pm, -1)
    p2 = p[ar, i2]

    # per-expert token lists (stable order)
    pairs = np.concatenate([i1, i2])
    toks = np.concatenate([ar, ar])
    wts = np.concatenate([p1, p2]).astype(f32)
    order = np.argsort(pairs, kind="stable")
    pairs_s, toks_s, wts_s = pairs[order], toks[order], wts[order]
    counts = np.bincount(pairs, minlength=E)
    starts = np.zeros(E + 1, np.int64)
    np.cumsum(counts, out=starts[1:])

    sel_tok = [None] * E
    sel_wt = [None] * E
    overflow = []
    for e in range(E):
        te = toks_s[starts[e]:starts[e + 1]]
        we = wts_s[starts[e]:starts[e + 1]]
        if len(te) > CAP:
            overflow.append((e, te[CAP:], we[CAP:]))
            te, we = te[:CAP], we[:CAP]
        sel_tok[e] = te
        sel_wt[e] = we

    use_b1 = bool(np.any(routed_b1))
    use_b2 = bool(np.any(routed_b2))
    use_bs1 = bool(np.any(shared_b1))
    nc = _get_nc((use_b1, use_b2, use_bs1, KDT))

    kdt = _np_dt(KDT)
    f8 = _np_dt("f8e3")
    ident_np = np.eye(128, dtype=kdt)
    xT = np.ascontiguousarray(x2.T)                      # [D, T] f32
    f8e4 = _np_dt("f8e4")
    xTk = xT.astype(kdt)                                 # bf16 for shared
    xT8 = _qc(xT, S_X, f8e4, 240.0)                      # e4m3 for routed fc1
    routed_w1k = np.stack([_qc(routed_w1[e], S_W1, f8e4, 240.0)
                           for e in range(E)])
    routed_w2k = np.stack([_qc(routed_w2[e], S_W2, f8, 15.5)
                           for e in range(E)])
    shared_w1k = shared_w1.astype(kdt)
    shared_w2k = np.stack([_qc(shared_w2[s], S_W2S, f8, 15.5)
                           for s in range(S)])

    in_maps = []
    for c in range(NCORE):
        es = [4 * c + i for i in range(EPC)]
        # gathered-padded tokens, one CAP-slot per expert
        idx_pad = np.zeros(EPC * CAP, np.int64)
        cw_pad = np.zeros((CAP, EPC), f32)
        for i, e in enumerate(es):
            n = len(sel_tok[e])
            idx_pad[i * CAP:i * CAP + n] = sel_tok[e]
            cw_pad[:n, i] = sel_wt[e]
        cw_pad *= S_OR / (S_A * S_W2)  # fp8 descales folded into combine wts
        xg_cols = xT8[:, idx_pad]  # [D, EPC*CAP] e4m3
        xg_np = np.stack([
            _pack_xT(xg_cols[:, i * CAP:(i + 1) * CAP]) for i in range(EPC)])
        w1p_np = np.stack([_pack_w1(routed_w1k[e]) for e in es])
        w2p_np = np.stack([_pack_w2(routed_w2k[e]) for e in es])

        s_c, q_c = c % S, c // S
        xq_np = _pack_xT(xTk[:, q_c * QT:(q_c + 1) * QT])
        w1sp_np = _pack_w1(shared_w1k[s_c])
        w2sp_np = _pack_w2(shared_w2k[s_c])

        im = {
            "xg": xg_np, "w1p": w1p_np, "w2p": w2p_np, "cwc": cw_pad,
            "xq": xq_np, "w1sp": w1sp_np, "w2sp": w2sp_np,
            "ident": ident_np,
        }
        if use_b1:
            im["b1r"] = (np.ascontiguousarray(routed_b1[es]) * S1).astype(kdt)
        if use_b2:
            im["b2r"] = (np.ascontiguousarray(routed_b2[es])
                         * (S_A * S_W2)).astype(kdt)
        if use_bs1:
            im["b1s"] = shared_b1[s_c:s_c + 1].astype(kdt)
        in_maps.append(im)

    res = run_bass_kernel_spmd(nc, in_maps, core_ids=list(range(NCORE)))

    # ---- host gather/unshard ----
    R = np.concatenate([np.asarray(res.results[c]["out_r"], np.float32)
                        for c in range(NCORE)], axis=0)
    R = R.reshape(E * CAP, D) * (1.0 / S_OR)
    tok_of_row = np.full(E * CAP, -1, np.int64)
    valid = np.zeros(E * CAP, bool)
    for e in range(E):
        n = len(sel_tok[e])
        tok_of_row[e * CAP:e * CAP + n] = sel_tok[e]
        valid[e * CAP:e * CAP + n] = True
    vrows = np.flatnonzero(valid)
    tv = tok_of_row[vrows]
    o = np.argsort(tv, kind="stable")
    out = np.zeros((T, D), f32)
    n_entries = np.bincount(tv, minlength=T)
    if n_entries.max() <= 2 and not overflow and n_entries.min() == 2:
        rows_sorted = vrows[o]
        out += R[rows_sorted[0::2]]
        out += R[rows_sorted[1::2]]
    else:
        np.add.at(out, tv, R[vrows])
    # overflow tokens: exact host fallback
    for e, te, we in overflow:
        xv = x2[te]
        h = xv @ routed_w1[e] + routed_b1[e]
        act = _silu(h[:, :F]) * h[:, F:]
        out[te] += we[:, None] * (act @ routed_w2[e] + routed_b2[e])

    # shared: quarters q handled by cores 2q (expert 0) and 2q+1 (expert 1)
    for q in range(NCORE // S):
        out[q * QT:(q + 1) * QT] += np.asarray(
            res.results[S * q]["out_s"], np.float32)
        out[q * QT:(q + 1) * QT] += np.asarray(
            res.results[S * q + 1]["out_s"], np.float32)
    out += shared_b2.sum(0)[None, :]

    return out.reshape(B, T, D).astype(f32)


# revision 20
# speedup vs baseline: 1.0320x; 1.0320x over previous
"""DeepSeekMoE Trainium2 kernel: 8-core expert-parallel sparse dispatch.

Strategy (hardcoded for D=5120, F=384, E=32, S=2, T=1024, top-2):
- Host computes the gate (softmax + top-2 + combine weights) and dispatches
  tokens: each of the 8 cores owns 4 routed experts; its assigned tokens are
  gathered, transposed and padded to 96 per expert on the host.
- Routed experts run in fp8 (x, w1 in e4m3 for DoubleRow 2x fc1; act + w2 in
  e3m4). The descales are folded into the per-token combine weights applied
  during the PSUM->SBUF copy. Routed outputs are written in e3m4 (x4 scale):
  they are attenuated ~4.6x by the combine weights, so the extra noise is
  negligible next to the bf16 shared outputs.
- Shared experts stay bf16 (their contribution dominates the output norm).
  Core c handles shared expert c%2 for token quarter c//2.
- Single global DMA read queue (sync engine) issues every load in exact
  consumption order, so early bandwidth goes to the first routed experts
  and the shared tensors stream in just-in-time. Writes go on gpsimd.
- Shared fc1 is split into NG groups interleaved between routed experts:
  each group accumulates 8 k-tiles in a transient PSUM tile (po ring) and
  folds into an SBUF f32 accumulator, so the tensor engine has work while
  routed weights stream and the tail only holds shared SwiGLU + fc2.
- Host gathers: routed outputs are scatter-added via two vectorized gathers,
  shared quarters are summed pairwise.
"""
import sys
import os

sys.path.insert(0, "/opt/trn_rl_repo")

import numpy as np

D = 5120
F = 384
F2 = 768
E = 32
S = 2
T = 1024
NCORE = 8
EPC = E // NCORE          # experts per core
CAP = 96                  # token capacity per expert slot (multiple of 32
                          # required by dual-fp8 ldweights; max load is 85)
QT = T // (NCORE // S)    # tokens per shared quarter = 256
DT = D // 128             # 40 d-tiles
G1 = 8                    # d-tiles per packed group (fc1 inputs)
NG = DT // G1             # 5 groups
FT = F // 128             # 3 f-tiles

KDT = "bf16"   # shared-expert compute dtype
# fp8 scales for the routed path: fc1 in e4m3 (DoubleRow 2x matmul rate),
# act + fc2 weights in e3m4 (better mantissa where it matters)
S_X = 8.0
S_W1 = 128.0
S_W2 = 100.0
S_A = 0.25
S_OR = 4.0     # routed-output e3m4 scale
S_W2S = 100.0  # shared fc2 weight e3m4 scale
S1 = S_X * S_W1
_compiled = {}


def _np_dt(name):
    import concourse.mybir as mybir
    m = {"bf16": mybir.dt.bfloat16, "f32r": mybir.dt.float32r,
         "f8e3": mybir.dt.float8e3, "f8e4": mybir.dt.float8e4}
    return mybir.dt.np(m[name])


def _build(use_b1, use_b2, use_bs1, kdt):
    import concourse.bass as bass
    import concourse.bacc as bacc
    import concourse.tile as tile
    import concourse.mybir as mybir

    F32 = mybir.dt.float32
    F32R = mybir.dt.bfloat16 if kdt == "bf16" else mybir.dt.float32r
    F8 = mybir.dt.float8e3
    F8E4 = mybir.dt.float8e4
    DR = mybir.MatmulPerfMode.DoubleRow
    AF = mybir.ActivationFunctionType

    nc = bacc.Bacc(None, target_bir_lowering=False)

    # ---- DRAM I/O ----
    # routed (fp8)
    xg = nc.dram_tensor("xg", [EPC, NG, 128, G1, CAP], F8E4,
                        kind="ExternalInput")
    w1p = nc.dram_tensor("w1p", [EPC, NG, 128, G1, F2], F8E4,
                         kind="ExternalInput")
    w2p = nc.dram_tensor("w2p", [EPC, FT, 128, D], F8, kind="ExternalInput")
    cwc = nc.dram_tensor("cwc", [CAP, EPC], F32, kind="ExternalInput")
    out_r = nc.dram_tensor("out_r", [EPC, CAP, D], F8, kind="ExternalOutput")
    # shared (bf16; this core's expert s=c%2, token quarter q=c//2)
    xq = nc.dram_tensor("xq", [NG, 128, G1, QT], F32R, kind="ExternalInput")
    w1sp = nc.dram_tensor("w1sp", [NG, 128, G1, F2], F32R, kind="ExternalInput")
    w2sp = nc.dram_tensor("w2sp", [FT, 128, D], F8, kind="ExternalInput")
    out_s = nc.dram_tensor("out_s", [QT, D], F32R, kind="ExternalOutput")
    # constants
    ident = nc.dram_tensor("ident", [128, 128], F32R, kind="ExternalInput")
    if use_b1:
        b1r = nc.dram_tensor("b1r", [EPC, F2], F32R, kind="ExternalInput")
    if use_b2:
        b2r = nc.dram_tensor("b2r", [EPC, D], F32R, kind="ExternalInput")
    if use_bs1:
        b1s = nc.dram_tensor("b1s", [1, F2], F32R, kind="ExternalInput")

    with tile.TileContext(nc) as tc:
        with (
            tc.tile_pool(name="cst", bufs=1) as cst,
            tc.tile_pool(name="res", bufs=1) as res,
            tc.tile_pool(name="wpool", bufs=5) as wpool,
            tc.tile_pool(name="w2pool", bufs=3) as w2pool,
            tc.tile_pool(name="w2spool", bufs=3) as w2spool,
            tc.tile_pool(name="xpool", bufs=5) as xpool,
            tc.tile_pool(name="spool", bufs=2) as spool,
            tc.tile_pool(name="opool", bufs=2) as opool,
            tc.tile_pool(name="ph", bufs=2, space="PSUM") as ph_pool,
            tc.tile_pool(name="pt", bufs=1, space="PSUM") as pt_pool,
            tc.tile_pool(name="po", bufs=3, space="PSUM") as po_pool,
        ):
            ident_t = cst.tile([128, 128], F32R)
            nc.gpsimd.dma_start(ident_t[:], ident[:])
            cw_t = cst.tile([CAP, EPC], F32)
            nc.gpsimd.dma_start(cw_t[:], cwc[:])
            need_ones = use_b1 or use_b2 or use_bs1
            if need_ones:
                ones_t = cst.tile([1, 128], F32R)
                nc.gpsimd.memset(ones_t[:], 1.0)
            if use_b1:
                b1r_t = cst.tile([EPC, F2], F32R)
                nc.gpsimd.dma_start(b1r_t[:], b1r[:])
            if use_b2:
                b2r_t = cst.tile([EPC, D], F32R)
                nc.gpsimd.dma_start(b2r_t[:], b2r[:])
            if use_bs1:
                b1s_t = cst.tile([1, F2], F32R)
                nc.gpsimd.dma_start(b1s_t[:], b1s[:])

            # resident shared-expert inputs (streamed just-in-time on the
            # global read queue) and the SBUF f32 fc1 accumulators
            xq_res = [res.tile([128, G1, QT], F32R, name=f"xqr{k}")
                      for k in range(NG)]
            w1s_res = [res.tile([128, G1, F2], F32R, name=f"w1sr{k}")
                       for k in range(NG)]
            acc_s = [res.tile([128, F2], F32, name=f"accs{t}")
                     for t in range(2)]

            def shared_load(k):
                nc.sync.dma_start(xq_res[k][:], xq[k])
                nc.sync.dma_start(w1s_res[k][:], w1sp[k])

            # ================= routed experts (fp8) =================
            def routed_fc1(e):
                psum_h = ph_pool.tile([CAP, F2], F32, tag="ph", padded_shape=None)
                if use_b1:
                    nc.tensor.matmul(psum_h[:, 0:512], ones_t[:, 0:CAP],
                                     b1r_t[e:e + 1, 0:512], start=True, stop=False)
                    nc.tensor.matmul(psum_h[:, 512:F2], ones_t[:, 0:CAP],
                                     b1r_t[e:e + 1, 512:F2], start=True, stop=False)
                for dtg in range(NG):
                    xg_t = xpool.tile([128, G1, CAP], F8E4, tag="xg")
                    nc.sync.dma_start(xg_t[:], xg[e, dtg])
                    w1_t = wpool.tile([128, G1, F2], F8E4, tag="w1")
                    nc.sync.dma_start(w1_t[:], w1p[e, dtg])
                    first = (dtg == 0) and not use_b1
                    last = dtg == NG - 1
                    for g in range(0, G1, 2):
                        nc.tensor.matmul(psum_h[:, 0:512],
                                         xg_t[:, g:g + 2, :],
                                         w1_t[:, g:g + 2, 0:512],
                                         start=first and g == 0,
                                         stop=(last and g == G1 - 2),
                                         perf_mode=DR)
                        nc.tensor.matmul(psum_h[:, 512:F2],
                                         xg_t[:, g:g + 2, :],
                                         w1_t[:, g:g + 2, 512:F2],
                                         start=first and g == 0,
                                         stop=(last and g == G1 - 2),
                                         perf_mode=DR)
                return psum_h

            def routed_w2_load(e):
                w2_t = [None] * FT
                for ft in range(FT):
                    w2_t[ft] = w2pool.tile([128, D], F8, tag="w2", name="w2t")
                    nc.sync.dma_start(w2_t[ft][:], w2p[e, ft])
                return w2_t

            def routed_finish(e, psum_h, w2_t):
                # SwiGLU: silu(v/S1) * (g*S1), bf16 transpose, then the
                # S_A/S1 descale + e3m4 cast fused into the PSUM->SBUF copy.
                silu_t = spool.tile([128, F], F32, tag="silu")
                nc.scalar.activation(silu_t[:CAP, :], psum_h[:, 0:F], AF.Silu,
                                     scale=1.0 / S1)
                act_t = spool.tile([128, F], F32R, tag="act")
                nc.vector.tensor_mul(act_t[:CAP, :], silu_t[:CAP, :],
                                     psum_h[:, F:F2])
                actT = spool.tile([128, FT, 128], F8, tag="actT")
                for ft in range(FT):
                    ptile = pt_pool.tile([128, 128], F32R)
                    nc.tensor.transpose(
                        ptile[:, 0:CAP], act_t[:CAP, ft * 128:(ft + 1) * 128],
                        ident_t[:CAP, 0:CAP]
                    )
                    nc.scalar.activation(actT[:, ft, 0:CAP], ptile[:, 0:CAP],
                                         AF.Copy, scale=S_A / S1)
                ob = opool.tile([128, D], F8, tag="ob")
                for ch in range(10):
                    po = po_pool.tile([CAP, 512], F32, tag="po", name="po")
                    if use_b2:
                        nc.tensor.matmul(
                            po[:], ones_t[:, 0:CAP],
                            b2r_t[e:e + 1, ch * 512:(ch + 1) * 512],
                            start=True, stop=False)
                    for ft in range(FT):
                        nc.tensor.matmul(
                            po[:], actT[:, ft, 0:CAP],
                            w2_t[ft][:, ch * 512:(ch + 1) * 512],
                            start=(ft == 0) and not use_b2,
                            stop=(ft == FT - 1))
                    nc.scalar.activation(
                        ob[:CAP, ch * 512:(ch + 1) * 512],
                        po[:], AF.Copy, scale=cw_t[:, e:e + 1])
                nc.gpsimd.dma_start(out_r[e], ob[:CAP, :])

            # ====== shared expert fc1, one k-group at a time (bf16) ======
            def shared_fc1_group(dtg):
                # accumulate 8 k-tiles into transient PSUM (po ring), then
                # fold into the SBUF f32 accumulator on vector/scalar
                for tt in range(2):
                    pa = po_pool.tile([128, 512], F32, tag="po", name="po")
                    pb = po_pool.tile([128, 256], F32, tag="po", name="po")
                    if use_bs1 and dtg == 0:
                        nc.tensor.matmul(pa[:], ones_t[:, 0:128],
                                         b1s_t[:, 0:512], start=True, stop=False)
                        nc.tensor.matmul(pb[:], ones_t[:, 0:128],
                                         b1s_t[:, 512:F2], start=True, stop=False)
                    for g in range(G1):
                        st = (g == 0) and not (use_bs1 and dtg == 0)
                        nc.tensor.matmul(
                            pa[:],
                            xq_res[dtg][:, g, tt * 128:(tt + 1) * 128],
                            w1s_res[dtg][:, g, 0:512],
                            start=st, stop=(g == G1 - 1))
                        nc.tensor.matmul(
                            pb[:],
                            xq_res[dtg][:, g, tt * 128:(tt + 1) * 128],
                            w1s_res[dtg][:, g, 512:F2],
                            start=st, stop=(g == G1 - 1))
                    if dtg == 0:
                        nc.vector.tensor_copy(acc_s[tt][:, 0:512], pa[:])
                        nc.scalar.activation(acc_s[tt][:, 512:F2], pb[:],
                                             AF.Copy)
                    else:
                        nc.vector.tensor_add(acc_s[tt][:, 0:512],
                                             acc_s[tt][:, 0:512], pa[:])
                        nc.vector.tensor_add(acc_s[tt][:, 512:F2],
                                             acc_s[tt][:, 512:F2], pb[:])

            # ============ shared expert SwiGLU + fc2 (tail) ============
            def shared_tail():
                w2s_t = [None] * FT
                for ft in range(FT):
                    w2s_t[ft] = w2spool.tile([128, D], F8, tag="w2s",
                                             name="w2st")
                    nc.sync.dma_start(w2s_t[ft][:], w2sp[ft])
                actTs = []
                for tt in range(2):
                    silu_t = spool.tile([128, F], F32, tag="silu")
                    nc.scalar.activation(silu_t[:, :], acc_s[tt][:, 0:F],
                                         AF.Silu)
                    act_t = spool.tile([128, F], F32R, tag="acts")
                    nc.vector.tensor_mul(act_t[:, :], silu_t[:, :],
                                         acc_s[tt][:, F:F2])
                    actT = spool.tile([128, FT, 128], F32R, tag="actTs")
                    for ft in range(FT):
                        ptile = pt_pool.tile([128, 128], F32R)
                        nc.tensor.transpose(
                            ptile[:, :], act_t[:, ft * 128:(ft + 1) * 128],
                            ident_t[:, :]
                        )
                        if ft % 2 == 0:
                            nc.vector.tensor_copy(actT[:, ft, :], ptile[:, :])
                        else:
                            nc.scalar.activation(actT[:, ft, :], ptile[:, :],
                                                 AF.Copy)
                    actTs.append(actT)
                obs = [opool.tile([128, D], F32R, tag="ob", name="obs")
                       for _ in range(2)]
                for tt in range(2):
                    for ch in range(10):
                        po = po_pool.tile([128, 512], F32, tag="po", name="po")
                        for ft in range(FT):
                            nc.tensor.matmul(
                                po[:], actTs[tt][:, ft, :],
                                w2s_t[ft][:, ch * 512:(ch + 1) * 512],
                                start=(ft == 0), stop=(ft == FT - 1))
                        # alternate copy engine so neither backs up at the
                        # tail; fold the w2s e3m4 descale into the copy
                        if ch % 2 == 0:
                            nc.vector.tensor_scalar_mul(
                                obs[tt][:, ch * 512:(ch + 1) * 512], po[:],
                                1.0 / S_W2S)
                        else:
                            nc.scalar.activation(
                                obs[tt][:, ch * 512:(ch + 1) * 512], po[:],
                                AF.Copy, scale=1.0 / S_W2S)
                        if ch == 4:
                            nc.gpsimd.dma_start(
                                out_s[tt * 128:(tt + 1) * 128, 0:2560],
                                obs[tt][:, 0:2560])
                        elif ch == 7:
                            nc.gpsimd.dma_start(
                                out_s[tt * 128:(tt + 1) * 128, 2560:4096],
                                obs[tt][:, 2560:4096])
                    nc.gpsimd.dma_start(
                        out_s[tt * 128:(tt + 1) * 128, 4096:D],
                        obs[tt][:, 4096:D])

            # Software pipeline. All reads are issued on the sync queue in
            # exact consumption order; shared fc1 groups fill the tensor
            # slack while routed weights stream.
            ph = [None] * EPC
            ph[0] = routed_fc1(0)
            ph[1] = routed_fc1(1)
            w2_0 = routed_w2_load(0)
            shared_load(0)
            routed_finish(0, ph[0], w2_0)
            shared_fc1_group(0)
            ph[2] = routed_fc1(2)
            w2_1 = routed_w2_load(1)
            shared_load(1)
            routed_finish(1, ph[1], w2_1)
            shared_fc1_group(1)
            ph[3] = routed_fc1(3)
            w2_2 = routed_w2_load(2)
            shared_load(2)
            routed_finish(2, ph[2], w2_2)
            shared_fc1_group(2)
            w2_3 = routed_w2_load(3)
            shared_load(3)
            routed_finish(3, ph[3], w2_3)
            shared_fc1_group(3)
            shared_load(4)
            shared_fc1_group(4)
            shared_tail()
    nc.compile()
    return nc


def _get_nc(key):
    if key not in _compiled:
        _compiled[key] = _build(*key)
    return _compiled[key]


def _silu(v):
    return v / (1.0 + np.exp(-v))


def _pack_w1(w):  # [D, 2F] -> [NG, 128, G1, 2F]
    return np.ascontiguousarray(
        w.reshape(NG, G1, 128, F2).transpose(0, 2, 1, 3))


def _pack_w2(w):  # [F, D] -> [FT, 128, D]
    return np.ascontiguousarray(w.reshape(FT, 128, D))


def _pack_xT(xt_cols):  # [D, ncols] -> [NG, 128, G1, ncols]
    n = xt_cols.shape[1]
    return np.ascontiguousarray(
        xt_cols.reshape(NG, G1, 128, n).transpose(0, 2, 1, 3))


def _qc(a, s, dt, lim):
    return np.clip(np.asarray(a, np.float32) * s, -lim, lim).astype(dt)


def kernel(x, gate_w, gate_b, shared_w1, shared_b1, shared_w2, shared_b2,
           routed_w1, routed_b1, routed_w2, routed_b2):
    from concourse.bass_utils import run_bass_kernel_spmd

    f32 = np.float32
    x = np.asarray(x, f32)
    gate_w = np.asarray(gate_w, f32)
    gate_b = np.asarray(gate_b, f32)
    shared_w1 = np.asarray(shared_w1, f32)
    shared_b1 = np.asarray(shared_b1, f32)
    shared_w2 = np.asarray(shared_w2, f32)
    shared_b2 = np.asarray(shared_b2, f32)
    routed_w1 = np.asarray(routed_w1, f32)
    routed_b1 = np.asarray(routed_b1, f32)
    routed_w2 = np.asarray(routed_w2, f32)
    routed_b2 = np.asarray(routed_b2, f32)

    B = x.shape[0]
    x2 = x.reshape(T, D)

    # ---- gate: softmax + top-2 (unnormalized combine weights) ----
    logits = x2 @ gate_w + gate_b
    m = logits.max(-1, keepdims=True)
    p = np.exp(logits - m, dtype=f32)
    p = p / p.sum(-1, keepdims=True)
    ar = np.arange(T)
    i1 = np.argmax(p, -1)
    p1 = p[ar, i1]
    pm = p.copy()
    pm[ar, i1] = -1.0
    i2 = np.argmax(pm, -1)
    p2 = p[ar, i2]

    # per-expert token lists (stable order)
    pairs = np.concatenate([i1, i2])
    toks = np.concatenate([ar, ar])
    wts = np.concatenate([p1, p2]).astype(f32)
    order = np.argsort(pairs, kind="stable")
    pairs_s, toks_s, wts_s = pairs[order], toks[order], wts[order]
    counts = np.bincount(pairs, minlength=E)
    starts = np.zeros(E + 1, np.int64)
    np.cumsum(counts, out=starts[1:])

    sel_tok = [None] * E
    sel_wt = [None] * E
    overflow = []
    for e in range(E):
        te = toks_s[starts[e]:starts[e + 1]]
        we = wts_s[starts[e]:starts[e + 1]]
        if len(te) > CAP:
            overflow.append((e, te[CAP:], we[CAP:]))
            te, we = te[:CAP], we[:CAP]
        sel_tok[e] = te
        sel_wt[e] = we

    use_b1 = bool(np.any(routed_b1))
    use_b2 = bool(np.any(routed_b2))
    use_bs1 = bool(np.any(shared_b1))
    nc = _get_nc((use_b1, use_b2, use_bs1, KDT))

    kdt = _np_dt(KDT)
    f8 = _np_dt("f8e3")
    ident_np = np.eye(128, dtype=kdt)
    xT = np.ascontiguousarray(x2.T)                      # [D, T] f32
    f8e4 = _np_dt("f8e4")
    xTk = xT.astype(kdt)                                 # bf16 for shared
    xT8 = _qc(xT, S_X, f8e4, 240.0)                      # e4m3 for routed fc1
    routed_w1k = np.stack([_qc(routed_w1[e], S_W1, f8e4, 240.0)
                           for e in range(E)])
    routed_w2k = np.stack([_qc(routed_w2[e], S_W2, f8, 15.5)
                           for e in range(E)])
    shared_w1k = shared_w1.astype(kdt)
    shared_w2k = np.stack([_qc(shared_w2[s], S_W2S, f8, 15.5)
                           for s in range(S)])

    in_maps = []
    for c in range(NCORE):
        es = [4 * c + i for i in range(EPC)]
        # gathered-padded tokens, one CAP-slot per expert
        idx_pad = np.zeros(EPC * CAP, np.int64)
        cw_pad = np.zeros((CAP, EPC), f32)
        for i, e in enumerate(es):
            n = len(sel_tok[e])
            idx_pad[i * CAP:i * CAP + n] = sel_tok[e]
            cw_pad[:n, i] = sel_wt[e]
        cw_pad *= S_OR / (S_A * S_W2)  # fp8 descales folded into combine wts
        xg_cols = xT8[:, idx_pad]  # [D, EPC*CAP] e4m3
        xg_np = np.stack([
            _pack_xT(xg_cols[:, i * CAP:(i + 1) * CAP]) for i in range(EPC)])
        w1p_np = np.stack([_pack_w1(routed_w1k[e]) for e in es])
        w2p_np = np.stack([_pack_w2(routed_w2k[e]) for e in es])

        s_c, q_c = c % S, c // S
        xq_np = _pack_xT(xTk[:, q_c * QT:(q_c + 1) * QT])
        w1sp_np = _pack_w1(shared_w1k[s_c])
        w2sp_np = _pack_w2(shared_w2k[s_c])

        im = {
            "xg": xg_np, "w1p": w1p_np, "w2p": w2p_np, "cwc": cw_pad,
            "xq": xq_np, "w1sp": w1sp_np, "w2sp": w2sp_np,
            "ident": ident_np,
        }
        if use_b1:
            im["b1r"] = (np.ascontiguousarray(routed_b1[es]) * S1).astype(kdt)
        if use_b2:
            im["b2r"] = (np.ascontiguousarray(routed_b2[es])
                         * (S_A * S_W2)).astype(kdt)
        if use_bs1:
            im["b1s"] = shared_b1[s_c:s_c + 1].astype(kdt)
        in_maps.append(im)

    res = run_bass_kernel_spmd(nc, in_maps, core_ids=list(range(NCORE)))

    # ---- host gather/unshard ----
    R = np.concatenate([np.asarray(res.results[c]["out_r"], np.float32)
                        for c in range(NCORE)], axis=0)
    R = R.reshape(E * CAP, D) * (1.0 / S_OR)
    tok_of_row = np.full(E * CAP, -1, np.int64)
    valid = np.zeros(E * CAP, bool)
    for e in range(E):
        n = len(sel_tok[e])
        tok_of_row[e * CAP:e * CAP + n] = sel_tok[e]
        valid[e * CAP:e * CAP + n] = True
    vrows = np.flatnonzero(valid)
    tv = tok_of_row[vrows]
    o = np.argsort(tv, kind="stable")
    out = np.zeros((T, D), f32)
    n_entries = np.bincount(tv, minlength=T)
    if n_entries.max() <= 2 and not overflow and n_entries.min() == 2:
        rows_sorted = vrows[o]
        out += R[rows_sorted[0::2]]
        out += R[rows_sorted[1::2]]
    else:
        np.add.at(out, tv, R[vrows])
    # overflow tokens: exact host fallback
    for e, te, we in overflow:
        xv = x2[te]
        h = xv @ routed_w1[e] + routed_b1[e]
        act = _silu(h[:, :F]) * h[:, F:]
        out[te] += we[:, None] * (act @ routed_w2[e] + routed_b2[e])

    # shared: quarters q handled by cores 2q (expert 0) and 2q+1 (expert 1)
    for q in range(NCORE // S):
        out[q * QT:(q + 1) * QT] += np.asarray(
            res.results[S * q]["out_s"], np.float32)
        out[q * QT:(q + 1) * QT] += np.asarray(
            res.results[S * q + 1]["out_s"], np.float32)
    out += shared_b2.sum(0)[None, :]

    return out.reshape(B, T, D).astype(f32)
